# revision 21
# baseline (speedup 1.0000x reference)
"""Sparse (per-query memory) attention kernel for 8 Trainium2 NeuronCores.

Problem shapes (hardcoded):
  x    [2, 8, 128, 512] f32
  mems [2, 8, 128, 64, 512] f32
  mask [2, 8, 128, 64] bool
  Wq [512, 512], Wkv [512, 1024], Wo [512, 512], bo [512]

Sharding: pure data-parallel over the 16 (b, m) slices -> 2 slices/core.

Restructured algorithm (eliminates the kv projection):
  p[i,h,:] = Wk_h @ (scale * Wq_h^T x[i])        (key-space query vector)
  sim[i,h,j] = mems[i,j,:] . p[i,h,:]
  attn = softmax_j(sim) with 0/1 mask applied post-exp
  mbar[i,h,:] = sum_j attn[i,h,j] * mems[i,j,:]
  out = (mbar . Wv_h per head) @ Wo + bo

v3 device mapping:
  - all projections computed directly in transposed orientation (no DMA
    transposes); attn / mbar flipped via TensorE transpose-mode.
  - scores: fp8 DoubleRow outer-product per 16-query block, psum/stationary
    rows ordered (h, iq); per-ib-pair mt8 chunks so scores start early.
  - score block-diagonal extracted with 16 partition-strided SBUF->SBUF
    DMAs per slice, split over the scalar HWDGE and gpsimd SWDGE queues
    (the sync ring carries only bulk input/output traffic).
  - softmax: one ACT exp, post-exp 0/1 mask multiply + Z reduce + recip.
  - mbar: block-diag stationary windows (pre-zeroed, persistent), 8
    accumulating [128x512] matmuls per 16-query group.
  - issue order A0 X0 A1 B0 X1 B1 keeps the PE queue dense: slice 1's
    scores overlap slice 0's softmax; slice 0's mbar overlaps slice 1's.
"""

import sys

sys.path.insert(0, "/opt/trn_rl_repo")

import numpy as np
import ml_dtypes

B, M, I, J = 2, 8, 128, 64
DIM, HEADS, DIM_HEAD = 512, 8, 64
INNER = HEADS * DIM_HEAD
SCALE = DIM_HEAD**-0.5
NCORES = 8
NSLICE = (B * M) // NCORES  # slices per core = 2
PS = 128.0  # fp8 p-vector pre-scale (keeps p out of e4m3 subnormals)

TRACE = False
last_results = None

_cache = {}


def _bc(ap, pos, count):
    """Insert a stride-0 (broadcast) dim of `count` at free position `pos`."""
    import concourse.bass as bass

    l = [list(d) for d in ap.ap]
    l.insert(pos, [0, count])
    return bass.AP(tensor=ap.tensor, offset=ap.offset, ap=l)


def _ap(ap, off, dims):
    """Custom AP on the same tensor: free dims [[step,count],...] after the
    partition dim, offset in elements relative to ap's offset."""
    import concourse.bass as bass

    l = [list(ap.ap[0])] + [list(d) for d in dims]
    return bass.AP(tensor=ap.tensor, offset=ap.offset + off, ap=l)


def _pstride(ap, p0, cnt, pstep, off, dims):
    """Partition-strided slice: partitions p0, p0+pstep, ... (cnt of them),
    plus free dims and element offset."""
    import concourse.bass as bass

    l = [list(d) for d in ap.ap]
    step = l[0][0]
    part = [step * pstep, cnt]
    return bass.AP(
        tensor=ap.tensor,
        offset=ap.offset + p0 * step + off,
        ap=[part] + [list(d) for d in dims],
    )


def bass_slice_part(ap, p0, cnt):
    """Slice partitions [p0, p0+cnt) of a 2D-ish AP."""
    import concourse.bass as bass

    l = [list(d) for d in ap.ap]
    step = l[0][0]
    l[0] = [step, cnt]
    return bass.AP(tensor=ap.tensor, offset=ap.offset + p0 * step, ap=l)


def _build():
    import concourse.tile as tile
    from concourse import bacc, mybir
    import concourse.bass as bass
    from contextlib import ExitStack

    f32 = mybir.dt.float32
    bf16 = mybir.dt.bfloat16
    f8 = mybir.dt.float8e4
    Exp = mybir.ActivationFunctionType.Exp
    DR = mybir.MatmulPerfMode.DoubleRow
    AxX = mybir.AxisListType.X

    nc = bacc.Bacc("TRN2", target_bir_lowering=False, debug=False, num_devices=NCORES)

    mt8_d = nc.dram_tensor("mt8", [NSLICE * 128, 32768], f8, kind="ExternalInput")
    mje_d = nc.dram_tensor("mje", [NSLICE * 128, 32768], bf16, kind="ExternalInput")
    xt_d = nc.dram_tensor("xt", [NSLICE * DIM, I], bf16, kind="ExternalInput")
    mk_d = nc.dram_tensor("mk", [NSLICE * 128, 512], bf16, kind="ExternalInput")
    wq_d = nc.dram_tensor("wq", [DIM, INNER], bf16, kind="ExternalInput")
    wkt_d = nc.dram_tensor("wkt", [128, 2048], bf16, kind="ExternalInput")
    wv_d = nc.dram_tensor("wv", [128, 2048], bf16, kind="ExternalInput")
    wo_d = nc.dram_tensor("wo", [INNER, DIM], bf16, kind="ExternalInput")
    bo_d = nc.dram_tensor("bo", [1, DIM], f32, kind="ExternalInput")
    id_d = nc.dram_tensor("ident", [128, 128], bf16, kind="ExternalInput")
    out_d = nc.dram_tensor("out", [NSLICE * I, DIM], f32, kind="ExternalOutput")

    with tile.TileContext(nc) as tc, ExitStack() as ctx:
        const = ctx.enter_context(tc.tile_pool(name="const", bufs=1))
        mt_pool = ctx.enter_context(tc.tile_pool(name="mt", bufs=1))
        mje_pool = ctx.enter_context(tc.tile_pool(name="mje", bufs=3))
        mbT_pool = ctx.enter_context(tc.tile_pool(name="mbT", bufs=1))
        scE_pool = ctx.enter_context(tc.tile_pool(name="scE", bufs=2))
        work = ctx.enter_context(tc.tile_pool(name="work", bufs=2))
        ps_sc = ctx.enter_context(tc.tile_pool(name="pssc", bufs=2, space="PSUM"))
        ps_mb = ctx.enter_context(tc.tile_pool(name="psmb", bufs=2, space="PSUM"))
        ps_misc = ctx.enter_context(tc.tile_pool(name="psmisc", bufs=2, space="PSUM"))
        ps_pT = ctx.enter_context(tc.tile_pool(name="pspT", bufs=2, space="PSUM"))

        # --- constant weights ---
        wq_sb = const.tile([128, 4, INNER], bf16)
        nc.sync.dma_start(out=wq_sb, in_=wq_d[:, :].rearrange("(c p) n -> p c n", p=128))
        wkt_sb = const.tile([128, 4, 512], bf16)
        nc.sync.dma_start(out=wkt_sb, in_=wkt_d[:, :].rearrange("p (c n) -> p c n", c=4))
        def load_late_weights():
            # deferred until after slice 0's inputs so scores start early
            wv_sb = const.tile([128, 4, 8, 64], bf16)
            nc.sync.dma_start(
                out=wv_sb, in_=wv_d[:, :].rearrange("p (c h n) -> p c h n", c=4, h=8)
            )
            wo_sb = const.tile([128, 4, DIM], bf16)
            nc.sync.dma_start(
                out=wo_sb, in_=wo_d[:, :].rearrange("(c p) n -> p c n", p=128)
            )
            bo_sb = const.tile([128, DIM], f32)
            nc.sync.dma_start(
                out=bo_sb,
                in_=_ap(bo_d[:, :], 0, [[1, DIM]]).to_broadcast([128, DIM]),
            )
            id_sb = const.tile([128, 128], bf16)
            nc.sync.dma_start(out=id_sb, in_=id_d[:, :])
            return wv_sb, wo_sb, bo_sb, id_sb

        # persistent zeroed at3 windows [128=(e2,j64), 8 l-windows * 128]
        at3s = []
        for g in range(8):
            t = const.tile([128, 1024], bf16, tag=f"at3_{g}")
            nc.gpsimd.memset(t, 0)
            at3s.append(t)

        st = [dict() for _ in range(NSLICE)]  # per-slice tile handles

        def phase_A(s):
            """input DMAs, q^T, p^T, scores -> scE."""
            d = st[s]
            xt_sb = work.tile([128, 4, I], bf16, tag="xt")
            nc.sync.dma_start(
                out=xt_sb,
                in_=xt_d[s * DIM : (s + 1) * DIM, :].rearrange(
                    "(c p) i -> p c i", p=128
                ),
            )
            mk_sb = work.tile([128, 512], bf16, tag="mk")
            nc.sync.dma_start(out=mk_sb, in_=mk_d[s * 128 : (s + 1) * 128, :])
            d["mk"] = mk_sb
            mt8c = []
            for c in range(4):
                t = mt_pool.tile([128, 8192], f8, tag=f"mt8_{c}")
                nc.sync.dma_start(
                    out=t,
                    in_=mt8_d[s * 128 : (s + 1) * 128, c * 8192 : (c + 1) * 8192],
                )
                mt8c.append(t)

            # q^T directly: qt[(hp,dh), (c,i)]
            qt_ps = ps_misc.tile([128, 512], f32, tag="misc")
            for c in range(4):
                for dc in range(4):
                    nc.tensor.matmul(
                        qt_ps[:, c * 128 : (c + 1) * 128],
                        wq_sb[:, dc, c * 128 : (c + 1) * 128],
                        xt_sb[:, dc, :],
                        start=(dc == 0),
                        stop=(dc == 3),
                    )
            qt_sb = work.tile([128, 4, I], bf16, tag="qt")
            nc.vector.tensor_copy(
                out=qt_sb[:, :, :].rearrange("p a b -> p (a b)"), in_=qt_ps
            )

            # p^T = (Wk_h * PS) @ q_h, cast fp8. pt8 free: (dc4, ib8, h8, iq16)
            pt8 = work.tile([128, 4096], f8, tag="pt8")
            for h in range(8):
                pp = ps_misc.tile([128, 512], f32, tag="misc")
                hp = h % 2
                for dc in range(4):
                    nc.tensor.matmul(
                        pp[:, dc * 128 : (dc + 1) * 128],
                        wkt_sb[
                            hp * 64 : hp * 64 + 64, h // 2, dc * 128 : (dc + 1) * 128
                        ],
                        qt_sb[hp * 64 : hp * 64 + 64, h // 2, :],
                        start=True,
                        stop=True,
                    )
                # dst col = dc*1024 + ib*128 + h*16 + iq  (contiguous 16-runs)
                dst = _ap(pt8[:, :], h * 16, [[1024, 4], [128, 8], [1, 16]])
                src = _ap(pp[:, :], 0, [[128, 4], [16, 8], [1, 16]])
                if h % 2 == 0:
                    nc.vector.tensor_copy(out=dst, in_=src)
                else:
                    nc.scalar.copy(out=dst, in_=src)

            # scores (fp8 DoubleRow): psum rows (h8, iq16), hf-major so the
            # diag extraction of each half starts at half-time.
            # scE half-tile hf: [128, (ib8, j64x8q)]; extraction DMAs inline.
            sim = work.tile([128, 512], f32, tag="sim")
            d["sim"] = sim
            for hf in range(2):
                scE = scE_pool.tile([128, 4096], f32, tag=f"scE{hf}")
                for ib in range(8):
                    ibp, e = ib // 2, ib % 2
                    sc = ps_sc.tile([128, 512], f32, tag="sc")
                    for p in range(2):
                        stat = _ap(
                            pt8[:, :], p * 2048 + ib * 128, [[1024, 2], [1, 128]]
                        )
                        mov = _ap(
                            mt8c[ibp][:, :],
                            p * 4096 + e * 1024 + hf * 512,
                            [[2048, 2], [1, 512]],
                        )
                        nc.tensor.matmul(
                            sc,
                            stat,
                            mov,
                            start=(p == 0),
                            stop=(p == 1),
                            perf_mode=DR,
                            skip_group_check=True,
                        )
                    dst = scE[:, ib * 512 : (ib + 1) * 512]
                    if ib % 3 == 2:
                        nc.scalar.copy(out=dst, in_=sc)
                    else:
                        nc.vector.tensor_copy(out=dst, in_=sc)
                # extraction: queries iq = hf*8 + q, q in [0,8)
                for q in range(8):
                    iq = hf * 8 + q
                    src = _pstride(
                        scE[:, :], iq, 8, 16, q * 64, [[512, 8], [1, 64]]
                    )
                    dstx = _pstride(sim[:, :], iq, 8, 16, 0, [[64, 8], [1, 64]])
                    eng = (nc.scalar, nc.gpsimd, nc.sync)[q % 3]
                    eng.dma_start(out=dstx, in_=src)

        def phase_X(s):
            """softmax (extraction DMAs were issued inline in phase A)."""
            d = st[s]
            sim, mk_sb = d["sim"], d["mk"]
            ex = work.tile([128, 512], bf16, tag="ex")
            nc.scalar.activation(out=ex, in_=sim, func=Exp, bias=0.0, scale=1.0 / PS)
            em = work.tile([128, 8, 64], bf16, tag="em")
            nc.vector.tensor_mul(
                em[:, :, :].rearrange("p a b -> p (a b)"), ex, mk_sb
            )
            zs = work.tile([128, 8], f32, tag="zs")
            nc.vector.reduce_sum(zs, em, axis=AxX)
            rz = work.tile([128, 8], f32, tag="rz")
            nc.vector.reciprocal(rz, zs)
            rzb = work.tile([128, 8], bf16, tag="rzb")
            nc.vector.tensor_copy(out=rzb, in_=rz)
            attn = work.tile([128, 8, 64], bf16, tag="attn")
            nc.vector.tensor_mul(attn, em, _bc(rzb[:, :], 2, 64))
            d["attn"] = attn

        def phase_B(s):
            """attn^T, scatter, mbar(+mje DMA), mbar^T, o1, out."""
            d = st[s]
            attn = d["attn"]
            atT = ps_pT.tile([128, 512], bf16, tag="pT")
            for a in range(4):
                nc.tensor.transpose(
                    atT[:, a * 128 : (a + 1) * 128],
                    attn[:, 2 * a : 2 * a + 2, :].rearrange("p a b -> p (a b)"),
                    id_sb,
                )
            # scatter: src col = a*128 + h*16 + half*8 + l; dst l*144 + h*2 + e
            for g in range(8):
                a, half = g // 2, g % 2
                for e in range(2):
                    src = _ap(
                        bass_slice_part(atT[:, :], e * 64, 64),
                        a * 128 + half * 8,
                        [[1, 8], [16, 8]],
                    )
                    dst = _ap(
                        bass_slice_part(at3s[g][:, :], e * 64, 64),
                        e,
                        [[144, 8], [2, 8]],
                    )
                    nc.vector.tensor_copy(out=dst, in_=src)

            # mbar: psum partition p = l*16 + h*2 + e
            mbT_sb = mbT_pool.tile([128, 4, 1024], bf16, tag="mbT")
            for g in range(8):
                mje_sb = mje_pool.tile([128, 4096], bf16, tag="mje")
                nc.sync.dma_start(
                    out=mje_sb,
                    in_=mje_d[s * 128 : (s + 1) * 128, g * 4096 : (g + 1) * 4096],
                )
                mb = ps_mb.tile([128, 512], f32, tag="mb")
                for l in range(8):
                    nc.tensor.matmul(
                        mb,
                        at3s[g][:, l * 128 : l * 128 + 128],
                        mje_sb[:, l * 512 : (l + 1) * 512],
                        start=(l == 0),
                        stop=(l == 7),
                    )
                mbE = work.tile([128, 512], bf16, tag="mbE")
                if g % 2 == 0:
                    nc.vector.tensor_copy(out=mbE, in_=mb)
                else:
                    nc.scalar.copy(out=mbE, in_=mb)
                mbT_ps = ps_pT.tile([128, 512], bf16, tag="pT")
                for dc in range(4):
                    nc.tensor.transpose(
                        mbT_ps[:, dc * 128 : (dc + 1) * 128],
                        mbE[:, dc * 128 : (dc + 1) * 128],
                        id_sb,
                    )
                # dst col = dc*1024 + h*128 + g*16 + l*2 + e
                for e in range(2):
                    src = _ap(mbT_ps[:, :], e, [[128, 4], [16, 8], [2, 8]])
                    dst = _ap(
                        mbT_sb[:, :, :].rearrange("p a b -> p (a b)"),
                        g * 16 + e,
                        [[1024, 4], [2, 8], [128, 8]],
                    )
                    if (g + e) % 2 == 0:
                        nc.scalar.copy(out=dst, in_=src)
                    else:
                        nc.vector.tensor_copy(out=dst, in_=src)

            # o1^T: [(hp,dh), (c, m)]
            o1_ps = ps_misc.tile([128, 512], f32, tag="misc")
            for h in range(8):
                c, hp = h // 2, h % 2
                for dc in range(4):
                    nc.tensor.matmul(
                        o1_ps[hp * 64 : hp * 64 + 64, c * 128 : (c + 1) * 128],
                        wv_sb[:, dc, h, :],
                        mbT_sb[:, dc, h * 128 : (h + 1) * 128],
                        start=(dc == 0),
                        stop=(dc == 3),
                    )
            o1_sb = work.tile([128, 4, I], bf16, tag="o1")
            nc.vector.tensor_copy(
                out=o1_sb[:, :, :].rearrange("p a b -> p (a b)"), in_=o1_ps
            )

            fin = ps_mb.tile([128, DIM], f32, tag="mb")
            for c in range(4):
                nc.tensor.matmul(
                    fin, o1_sb[:, c, :], wo_sb[:, c, :], start=(c == 0), stop=(c == 3)
                )
            outb = work.tile([128, DIM], f32, tag="outb")
            nc.vector.tensor_add(outb, fin, bo_sb)
            nc.sync.dma_start(out=out_d[s * I : (s + 1) * I, :], in_=outb)

        phase_A(0)
        wv_sb, wo_sb, bo_sb, id_sb = load_late_weights()
        phase_X(0)
        phase_A(1)
        phase_X(1)
        phase_B(0)
        phase_B(1)

    nc.compile()
    return nc


def kernel(x, mems, mask, Wq, Wkv, Wo, bo):
    from concourse.bass_utils import run_bass_kernel_spmd

    global last_results

    if "nc" not in _cache:
        _cache["nc"] = _build()
    nc = _cache["nc"]

    bf = ml_dtypes.bfloat16
    f8 = ml_dtypes.float8_e4m3
    S = B * M
    x = np.asarray(x, dtype=np.float32).reshape(S, I, DIM)
    mems = np.asarray(mems, dtype=np.float32).reshape(S, I, J, DIM)
    mask = np.asarray(mask).reshape(S, I, J)

    # memsT8 fp8: [dsub128, ibp4, pass2, k2, e2, iq16, j64]
    mt8 = (
        mems.astype(f8)
        .reshape(S, 4, 2, 16, J, 2, 2, 128)  # [S, ibp, e, iq, j, pass, k, dsub]
        .transpose(0, 7, 1, 5, 6, 2, 3, 4)  # [S, dsub, ibp, pass, k, e, iq, j]
        .reshape(S, 128, 32768)
    )
    # memsJE bf16 rows (e2,j64), cols (a4, half2, l8, d): i = a*32+e*16+half*8+l
    mje = (
        mems.astype(bf)
        .reshape(S, 4, 2, 2, 8, J, DIM)  # [S, a, e, half, l, j, d]
        .transpose(0, 2, 5, 1, 3, 4, 6)  # [S, e, j, a, half, l, d]
        .reshape(S, 128, 32768)
    )
    # xT bf16 [d, i]
    xt = np.ascontiguousarray(x.transpose(0, 2, 1)).astype(bf)
    # mask 0/1 bf16: rows (h8, iq16), cols (ib8, j64)
    m1 = mask.reshape(S, 8, 16, J).transpose(0, 2, 1, 3)  # [S, iq, ib, j]
    mk = np.broadcast_to(m1[:, None, :, :, :], (S, 8, 16, 8, J)).reshape(S, 128, 512)
    mk = np.ascontiguousarray(mk).astype(bf)

    Wq_r = (np.asarray(Wq, np.float32) * SCALE).astype(bf)
    Wkv = np.asarray(Wkv, np.float32)
    Wk, Wv = Wkv[:, :INNER], Wkv[:, INNER:]
    # WkT xPS: [(hp2, dh64), c4, D512]: row hp*64+dh, chunk c -> head 2c+hp
    wkt = (
        (Wk * PS)
        .reshape(DIM, 4, 2, 64)  # [D, c, hp, dh]
        .transpose(2, 3, 1, 0)  # [hp, dh, c, D]
        .reshape(128, 2048)
    )
    wkt = np.ascontiguousarray(wkt).astype(bf)
    # Wv: [dsub128, dc4, h8, dh64]
    wv = np.ascontiguousarray(
        Wv.reshape(4, 128, 8, 64).transpose(1, 0, 2, 3)
    ).astype(bf).reshape(128, 2048)
    wo_r = np.asarray(Wo, np.float32).astype(bf)
    bo_r = np.asarray(bo, np.float32).reshape(1, DIM)
    ident = np.eye(128, dtype=bf)

    in_maps = []
    for c in range(NCORES):
        sl = slice(NSLICE * c, NSLICE * (c + 1))
        in_maps.append(
            {
                "mt8": mt8[sl].reshape(NSLICE * 128, 32768),
                "mje": mje[sl].reshape(NSLICE * 128, 32768),
                "xt": xt[sl].reshape(NSLICE * DIM, I),
                "mk": mk[sl].reshape(NSLICE * 128, 512),
                "wq": Wq_r,
                "wkt": wkt,
                "wv": wv,
                "wo": wo_r,
                "bo": bo_r,
                "ident": ident,
            }
        )

    res = run_bass_kernel_spmd(nc, in_maps, core_ids=list(range(NCORES)), trace=TRACE)
    last_results = res

    # device row order m = g*16 + l*2 + e, g = a*2+half -> i = a*32+e*16+half*8+l
    mm = np.arange(128)
    g, le = mm // 16, mm % 16
    l, e = le // 2, le % 2
    a, half = g // 2, g % 2
    iperm = a * 32 + e * 16 + half * 8 + l  # iperm[m] = original query row i

    out = np.empty((S, I, DIM), np.float32)
    for c in range(NCORES):
        o = res.results[c]["out"].reshape(NSLICE, I, DIM)
        out[NSLICE * c : NSLICE * (c + 1), iperm, :] = o
    return out.reshape(B, M, I, DIM)


# revision 23
# speedup vs baseline: 1.0109x; 1.0109x over previous
"""Sparse (per-query memory) attention kernel for 8 Trainium2 NeuronCores.

Problem shapes (hardcoded):
  x    [2, 8, 128, 512] f32
  mems [2, 8, 128, 64, 512] f32
  mask [2, 8, 128, 64] bool
  Wq [512, 512], Wkv [512, 1024], Wo [512, 512], bo [512]

Sharding: pure data-parallel over the 16 (b, m) slices -> 2 slices/core.

Restructured algorithm (eliminates the kv projection):
  p[i,h,:] = Wk_h @ (scale * Wq_h^T x[i])        (key-space query vector)
  sim[i,h,j] = mems[i,j,:] . p[i,h,:]
  attn = softmax_j(sim) with 0/1 mask applied post-exp
  mbar[i,h,:] = sum_j attn[i,h,j] * mems[i,j,:]
  out = (mbar . Wv_h per head) @ Wo + bo

v3 device mapping:
  - all projections computed directly in transposed orientation (no DMA
    transposes); attn / mbar flipped via TensorE transpose-mode.
  - scores: fp8 DoubleRow outer-product per 16-query block, psum/stationary
    rows ordered (h, iq); per-ib-pair mt8 chunks so scores start early.
  - score block-diagonal extracted with 16 partition-strided SBUF->SBUF
    DMAs per slice, split over the scalar HWDGE and gpsimd SWDGE queues
    (the sync ring carries only bulk input/output traffic).
  - softmax: one ACT exp, post-exp 0/1 mask multiply + Z reduce + recip.
  - mbar: block-diag stationary windows (pre-zeroed, persistent), 8
    accumulating [128x512] matmuls per 16-query group.
  - issue order A0 X0 A1 B0 X1 B1 keeps the PE queue dense: slice 1's
    scores overlap slice 0's softmax; slice 0's mbar overlaps slice 1's.
"""

import sys

sys.path.insert(0, "/opt/trn_rl_repo")

import numpy as np
import ml_dtypes

B, M, I, J = 2, 8, 128, 64
DIM, HEADS, DIM_HEAD = 512, 8, 64
INNER = HEADS * DIM_HEAD
SCALE = DIM_HEAD**-0.5
NCORES = 8
NSLICE = (B * M) // NCORES  # slices per core = 2
PS = 128.0  # fp8 p-vector pre-scale (keeps p out of e4m3 subnormals)

TRACE = False
last_results = None

_cache = {}


def _bc(ap, pos, count):
    """Insert a stride-0 (broadcast) dim of `count` at free position `pos`."""
    import concourse.bass as bass

    l = [list(d) for d in ap.ap]
    l.insert(pos, [0, count])
    return bass.AP(tensor=ap.tensor, offset=ap.offset, ap=l)


def _ap(ap, off, dims):
    """Custom AP on the same tensor: free dims [[step,count],...] after the
    partition dim, offset in elements relative to ap's offset."""
    import concourse.bass as bass

    l = [list(ap.ap[0])] + [list(d) for d in dims]
    return bass.AP(tensor=ap.tensor, offset=ap.offset + off, ap=l)


def _pstride(ap, p0, cnt, pstep, off, dims):
    """Partition-strided slice: partitions p0, p0+pstep, ... (cnt of them),
    plus free dims and element offset."""
    import concourse.bass as bass

    l = [list(d) for d in ap.ap]
    step = l[0][0]
    part = [step * pstep, cnt]
    return bass.AP(
        tensor=ap.tensor,
        offset=ap.offset + p0 * step + off,
        ap=[part] + [list(d) for d in dims],
    )


def bass_slice_part(ap, p0, cnt):
    """Slice partitions [p0, p0+cnt) of a 2D-ish AP."""
    import concourse.bass as bass

    l = [list(d) for d in ap.ap]
    step = l[0][0]
    l[0] = [step, cnt]
    return bass.AP(tensor=ap.tensor, offset=ap.offset + p0 * step, ap=l)


def _build():
    import concourse.tile as tile
    from concourse import bacc, mybir
    import concourse.bass as bass
    from contextlib import ExitStack

    f32 = mybir.dt.float32
    bf16 = mybir.dt.bfloat16
    f8 = mybir.dt.float8e4
    Exp = mybir.ActivationFunctionType.Exp
    DR = mybir.MatmulPerfMode.DoubleRow
    AxX = mybir.AxisListType.X

    nc = bacc.Bacc("TRN2", target_bir_lowering=False, debug=False, num_devices=NCORES)

    mt8_d = nc.dram_tensor("mt8", [NSLICE * 128, 32768], f8, kind="ExternalInput")
    mje_d = nc.dram_tensor("mje", [NSLICE * 128, 32768], bf16, kind="ExternalInput")
    xt_d = nc.dram_tensor("xt", [NSLICE * DIM, I], bf16, kind="ExternalInput")
    mk_d = nc.dram_tensor("mk", [NSLICE * 128, 512], bf16, kind="ExternalInput")
    wq_d = nc.dram_tensor("wq", [DIM, INNER], bf16, kind="ExternalInput")
    wkt_d = nc.dram_tensor("wkt", [128, 2048], bf16, kind="ExternalInput")
    wv_d = nc.dram_tensor("wv", [128, 2048], bf16, kind="ExternalInput")
    wo_d = nc.dram_tensor("wo", [INNER, DIM], bf16, kind="ExternalInput")
    bo_d = nc.dram_tensor("bo", [1, DIM], f32, kind="ExternalInput")
    id_d = nc.dram_tensor("ident", [128, 128], bf16, kind="ExternalInput")
    out_d = nc.dram_tensor("out", [NSLICE * I, DIM], f32, kind="ExternalOutput")

    with tile.TileContext(nc) as tc, ExitStack() as ctx:
        const = ctx.enter_context(tc.tile_pool(name="const", bufs=1))
        mt_pool = ctx.enter_context(tc.tile_pool(name="mt", bufs=1))
        mje_pool = ctx.enter_context(tc.tile_pool(name="mje", bufs=4))
        mbT_pool = ctx.enter_context(tc.tile_pool(name="mbT", bufs=1))
        scE_pool = ctx.enter_context(tc.tile_pool(name="scE", bufs=2))
        work = ctx.enter_context(tc.tile_pool(name="work", bufs=2))
        ps_sc = ctx.enter_context(tc.tile_pool(name="pssc", bufs=2, space="PSUM"))
        ps_mb = ctx.enter_context(tc.tile_pool(name="psmb", bufs=2, space="PSUM"))
        ps_misc = ctx.enter_context(tc.tile_pool(name="psmisc", bufs=2, space="PSUM"))
        ps_pT = ctx.enter_context(tc.tile_pool(name="pspT", bufs=2, space="PSUM"))

        # --- constant weights ---
        wq_sb = const.tile([128, 4, INNER], bf16)
        nc.sync.dma_start(out=wq_sb, in_=wq_d[:, :].rearrange("(c p) n -> p c n", p=128))
        wkt_sb = const.tile([128, 4, 512], bf16)
        nc.sync.dma_start(out=wkt_sb, in_=wkt_d[:, :].rearrange("p (c n) -> p c n", c=4))
        def load_late_weights():
            # deferred until after slice 0's inputs so scores start early
            wv_sb = const.tile([128, 4, 8, 64], bf16)
            nc.sync.dma_start(
                out=wv_sb, in_=wv_d[:, :].rearrange("p (c h n) -> p c h n", c=4, h=8)
            )
            wo_sb = const.tile([128, 4, DIM], bf16)
            nc.sync.dma_start(
                out=wo_sb, in_=wo_d[:, :].rearrange("(c p) n -> p c n", p=128)
            )
            bo_sb = const.tile([128, DIM], f32)
            nc.sync.dma_start(
                out=bo_sb,
                in_=_ap(bo_d[:, :], 0, [[1, DIM]]).to_broadcast([128, DIM]),
            )
            id_sb = const.tile([128, 128], bf16)
            nc.sync.dma_start(out=id_sb, in_=id_d[:, :])
            return wv_sb, wo_sb, bo_sb, id_sb

        # persistent zeroed at3 windows [128=(e2,j64), 8 l-windows * 128]
        at3s = []
        for g in range(8):
            t = const.tile([128, 1024], bf16, tag=f"at3_{g}")
            nc.gpsimd.memset(t, 0)
            at3s.append(t)

        st = [dict() for _ in range(NSLICE)]  # per-slice tile handles

        def phase_A(s):
            """input DMAs, q^T, p^T, scores -> scE."""
            d = st[s]
            xt_sb = work.tile([128, 4, I], bf16, tag="xt")
            nc.sync.dma_start(
                out=xt_sb,
                in_=xt_d[s * DIM : (s + 1) * DIM, :].rearrange(
                    "(c p) i -> p c i", p=128
                ),
            )
            mk_sb = work.tile([128, 512], bf16, tag="mk")
            nc.sync.dma_start(out=mk_sb, in_=mk_d[s * 128 : (s + 1) * 128, :])
            d["mk"] = mk_sb
            mt8c = []
            for c in range(4):
                t = mt_pool.tile([128, 8192], f8, tag=f"mt8_{c}")
                nc.sync.dma_start(
                    out=t,
                    in_=mt8_d[s * 128 : (s + 1) * 128, c * 8192 : (c + 1) * 8192],
                )
                mt8c.append(t)

            # q^T directly: qt[(hp,dh), (c,i)]
            qt_ps = ps_misc.tile([128, 512], f32, tag="misc")
            for c in range(4):
                for dc in range(4):
                    nc.tensor.matmul(
                        qt_ps[:, c * 128 : (c + 1) * 128],
                        wq_sb[:, dc, c * 128 : (c + 1) * 128],
                        xt_sb[:, dc, :],
                        start=(dc == 0),
                        stop=(dc == 3),
                    )
            qt_sb = work.tile([128, 4, I], bf16, tag="qt")
            nc.vector.tensor_copy(
                out=qt_sb[:, :, :].rearrange("p a b -> p (a b)"), in_=qt_ps
            )

            # p^T = (Wk_h * PS) @ q_h, cast fp8. pt8 free: (dc4, ib8, h8, iq16)
            pt8 = work.tile([128, 4096], f8, tag="pt8")
            for h in range(8):
                pp = ps_misc.tile([128, 512], f32, tag="misc")
                hp = h % 2
                for dc in range(4):
                    nc.tensor.matmul(
                        pp[:, dc * 128 : (dc + 1) * 128],
                        wkt_sb[
                            hp * 64 : hp * 64 + 64, h // 2, dc * 128 : (dc + 1) * 128
                        ],
                        qt_sb[hp * 64 : hp * 64 + 64, h // 2, :],
                        start=True,
                        stop=True,
                    )
                # dst col = dc*1024 + ib*128 + h*16 + iq  (contiguous 16-runs)
                dst = _ap(pt8[:, :], h * 16, [[1024, 4], [128, 8], [1, 16]])
                src = _ap(pp[:, :], 0, [[128, 4], [16, 8], [1, 16]])
                if h % 2 == 0:
                    nc.vector.tensor_copy(out=dst, in_=src)
                else:
                    nc.scalar.copy(out=dst, in_=src)

            # scores (fp8 DoubleRow): psum rows (h8, iq16), hf-major so the
            # diag extraction of each half starts at half-time.
            # scE half-tile hf: [128, (ib8, j64x8q)]; extraction DMAs inline.
            sim = work.tile([128, 512], f32, tag="sim")
            d["sim"] = sim
            for hf in range(2):
                scE = scE_pool.tile([128, 4096], f32, tag=f"scE{hf}")
                for ib in range(8):
                    ibp, e = ib // 2, ib % 2
                    sc = ps_sc.tile([128, 512], f32, tag="sc")
                    for p in range(2):
                        stat = _ap(
                            pt8[:, :], p * 2048 + ib * 128, [[1024, 2], [1, 128]]
                        )
                        mov = _ap(
                            mt8c[ibp][:, :],
                            p * 4096 + e * 1024 + hf * 512,
                            [[2048, 2], [1, 512]],
                        )
                        nc.tensor.matmul(
                            sc,
                            stat,
                            mov,
                            start=(p == 0),
                            stop=(p == 1),
                            perf_mode=DR,
                            skip_group_check=True,
                        )
                    dst = scE[:, ib * 512 : (ib + 1) * 512]
                    if ib % 3 == 2:
                        nc.scalar.copy(out=dst, in_=sc)
                    else:
                        nc.vector.tensor_copy(out=dst, in_=sc)
                # extraction: queries iq = hf*8 + q, q in [0,8)
                for q in range(8):
                    iq = hf * 8 + q
                    src = _pstride(
                        scE[:, :], iq, 8, 16, q * 64, [[512, 8], [1, 64]]
                    )
                    dstx = _pstride(sim[:, :], iq, 8, 16, 0, [[64, 8], [1, 64]])
                    eng = (nc.scalar, nc.gpsimd, nc.sync)[q % 3]
                    eng.dma_start(out=dstx, in_=src)

        def phase_X(s):
            """softmax (extraction DMAs were issued inline in phase A)."""
            d = st[s]
            sim, mk_sb = d["sim"], d["mk"]
            ex = work.tile([128, 512], bf16, tag="ex")
            nc.scalar.activation(out=ex, in_=sim, func=Exp, bias=0.0, scale=1.0 / PS)
            em = work.tile([128, 8, 64], bf16, tag="em")
            nc.vector.tensor_mul(
                em[:, :, :].rearrange("p a b -> p (a b)"), ex, mk_sb
            )
            zs = work.tile([128, 8], f32, tag="zs")
            nc.vector.reduce_sum(zs, em, axis=AxX)
            rz = work.tile([128, 8], f32, tag="rz")
            nc.vector.reciprocal(rz, zs)
            rzb = work.tile([128, 8], bf16, tag="rzb")
            nc.vector.tensor_copy(out=rzb, in_=rz)
            attn = work.tile([128, 8, 64], bf16, tag="attn")
            nc.vector.tensor_mul(attn, em, _bc(rzb[:, :], 2, 64))
            d["attn"] = attn

        def phase_B(s):
            """attn^T, scatter, mbar(+mje DMA), mbar^T, o1, out."""
            d = st[s]
            attn = d["attn"]
            atT = ps_pT.tile([128, 512], bf16, tag="pT")
            for a in range(4):
                nc.tensor.transpose(
                    atT[:, a * 128 : (a + 1) * 128],
                    attn[:, 2 * a : 2 * a + 2, :].rearrange("p a b -> p (a b)"),
                    id_sb,
                )
            # scatter: src col = a*128 + h*16 + half*8 + l; dst l*144 + h*2 + e
            for g in range(8):
                a, half = g // 2, g % 2
                for e in range(2):
                    src = _ap(
                        bass_slice_part(atT[:, :], e * 64, 64),
                        a * 128 + half * 8,
                        [[1, 8], [16, 8]],
                    )
                    dst = _ap(
                        bass_slice_part(at3s[g][:, :], e * 64, 64),
                        e,
                        [[144, 8], [2, 8]],
                    )
                    nc.vector.tensor_copy(out=dst, in_=src)

            # mbar: psum partition p = l*16 + h*2 + e
            mbT_sb = mbT_pool.tile([128, 4, 1024], bf16, tag="mbT")
            mjes = []
            for g in range(8):
                mje_sb = mje_pool.tile([128, 4096], bf16, tag="mje")
                nc.sync.dma_start(
                    out=mje_sb,
                    in_=mje_d[s * 128 : (s + 1) * 128, g * 4096 : (g + 1) * 4096],
                )
                mjes.append(mje_sb)
            for g in range(8):
                mje_sb = mjes[g]
                mb = ps_mb.tile([128, 512], f32, tag="mb")
                for l in range(8):
                    nc.tensor.matmul(
                        mb,
                        at3s[g][:, l * 128 : l * 128 + 128],
                        mje_sb[:, l * 512 : (l + 1) * 512],
                        start=(l == 0),
                        stop=(l == 7),
                    )
                mbE = work.tile([128, 512], bf16, tag="mbE")
                if g % 2 == 0:
                    nc.vector.tensor_copy(out=mbE, in_=mb)
                else:
                    nc.scalar.copy(out=mbE, in_=mb)
                mbT_ps = ps_pT.tile([128, 512], bf16, tag="pT")
                for dc in range(4):
                    nc.tensor.transpose(
                        mbT_ps[:, dc * 128 : (dc + 1) * 128],
                        mbE[:, dc * 128 : (dc + 1) * 128],
                        id_sb,
                    )
                # dst col = dc*1024 + h*128 + g*16 + l*2 + e
                for e in range(2):
                    src = _ap(mbT_ps[:, :], e, [[128, 4], [16, 8], [2, 8]])
                    dst = _ap(
                        mbT_sb[:, :, :].rearrange("p a b -> p (a b)"),
                        g * 16 + e,
                        [[1024, 4], [2, 8], [128, 8]],
                    )
                    if (g + e) % 2 == 0:
                        nc.scalar.copy(out=dst, in_=src)
                    else:
                        nc.vector.tensor_copy(out=dst, in_=src)

            # o1^T: [(hp,dh), (c, m)]
            o1_ps = ps_misc.tile([128, 512], f32, tag="misc")
            for h in range(8):
                c, hp = h // 2, h % 2
                for dc in range(4):
                    nc.tensor.matmul(
                        o1_ps[hp * 64 : hp * 64 + 64, c * 128 : (c + 1) * 128],
                        wv_sb[:, dc, h, :],
                        mbT_sb[:, dc, h * 128 : (h + 1) * 128],
                        start=(dc == 0),
                        stop=(dc == 3),
                    )
            o1_sb = work.tile([128, 4, I], bf16, tag="o1")
            nc.vector.tensor_copy(
                out=o1_sb[:, :, :].rearrange("p a b -> p (a b)"), in_=o1_ps
            )

            fin = ps_mb.tile([128, DIM], f32, tag="mb")
            for c in range(4):
                nc.tensor.matmul(
                    fin, o1_sb[:, c, :], wo_sb[:, c, :], start=(c == 0), stop=(c == 3)
                )
            outb = work.tile([128, DIM], f32, tag="outb")
            nc.vector.tensor_add(outb, fin, bo_sb)
            nc.sync.dma_start(out=out_d[s * I : (s + 1) * I, :], in_=outb)

        phase_A(0)
        wv_sb, wo_sb, bo_sb, id_sb = load_late_weights()
        phase_A(1)
        phase_X(0)
        phase_X(1)
        phase_B(0)
        phase_B(1)

    nc.compile()
    return nc


def kernel(x, mems, mask, Wq, Wkv, Wo, bo):
    from concourse.bass_utils import run_bass_kernel_spmd

    global last_results

    if "nc" not in _cache:
        _cache["nc"] = _build()
    nc = _cache["nc"]

    bf = ml_dtypes.bfloat16
    f8 = ml_dtypes.float8_e4m3
    S = B * M
    x = np.asarray(x, dtype=np.float32).reshape(S, I, DIM)
    mems = np.asarray(mems, dtype=np.float32).reshape(S, I, J, DIM)
    mask = np.asarray(mask).reshape(S, I, J)

    # memsT8 fp8: [dsub128, ibp4, pass2, k2, e2, iq16, j64]
    mt8 = (
        mems.astype(f8)
        .reshape(S, 4, 2, 16, J, 2, 2, 128)  # [S, ibp, e, iq, j, pass, k, dsub]
        .transpose(0, 7, 1, 5, 6, 2, 3, 4)  # [S, dsub, ibp, pass, k, e, iq, j]
        .reshape(S, 128, 32768)
    )
    # memsJE bf16 rows (e2,j64), cols (a4, half2, l8, d): i = a*32+e*16+half*8+l
    mje = (
        mems.astype(bf)
        .reshape(S, 4, 2, 2, 8, J, DIM)  # [S, a, e, half, l, j, d]
        .transpose(0, 2, 5, 1, 3, 4, 6)  # [S, e, j, a, half, l, d]
        .reshape(S, 128, 32768)
    )
    # xT bf16 [d, i]
    xt = np.ascontiguousarray(x.transpose(0, 2, 1)).astype(bf)
    # mask 0/1 bf16: rows (h8, iq16), cols (ib8, j64)
    m1 = mask.reshape(S, 8, 16, J).transpose(0, 2, 1, 3)  # [S, iq, ib, j]
    mk = np.broadcast_to(m1[:, None, :, :, :], (S, 8, 16, 8, J)).reshape(S, 128, 512)
    mk = np.ascontiguousarray(mk).astype(bf)

    Wq_r = (np.asarray(Wq, np.float32) * SCALE).astype(bf)
    Wkv = np.asarray(Wkv, np.float32)
    Wk, Wv = Wkv[:, :INNER], Wkv[:, INNER:]
    # WkT xPS: [(hp2, dh64), c4, D512]: row hp*64+dh, chunk c -> head 2c+hp
    wkt = (
        (Wk * PS)
        .reshape(DIM, 4, 2, 64)  # [D, c, hp, dh]
        .transpose(2, 3, 1, 0)  # [hp, dh, c, D]
        .reshape(128, 2048)
    )
    wkt = np.ascontiguousarray(wkt).astype(bf)
    # Wv: [dsub128, dc4, h8, dh64]
    wv = np.ascontiguousarray(
        Wv.reshape(4, 128, 8, 64).transpose(1, 0, 2, 3)
    ).astype(bf).reshape(128, 2048)
    wo_r = np.asarray(Wo, np.float32).astype(bf)
    bo_r = np.asarray(bo, np.float32).reshape(1, DIM)
    ident = np.eye(128, dtype=bf)

    in_maps = []
    for c in range(NCORES):
        sl = slice(NSLICE * c, NSLICE * (c + 1))
        in_maps.append(
            {
                "mt8": mt8[sl].reshape(NSLICE * 128, 32768),
                "mje": mje[sl].reshape(NSLICE * 128, 32768),
                "xt": xt[sl].reshape(NSLICE * DIM, I),
                "mk": mk[sl].reshape(NSLICE * 128, 512),
                "wq": Wq_r,
                "wkt": wkt,
                "wv": wv,
                "wo": wo_r,
                "bo": bo_r,
                "ident": ident,
            }
        )

    res = run_bass_kernel_spmd(nc, in_maps, core_ids=list(range(NCORES)), trace=TRACE)
    last_results = res

    # device row order m = g*16 + l*2 + e, g = a*2+half -> i = a*32+e*16+half*8+l
    mm = np.arange(128)
    g, le = mm // 16, mm % 16
    l, e = le // 2, le % 2
    a, half = g // 2, g % 2
    iperm = a * 32 + e * 16 + half * 8 + l  # iperm[m] = original query row i

    out = np.empty((S, I, DIM), np.float32)
    for c in range(NCORES):
        o = res.results[c]["out"].reshape(NSLICE, I, DIM)
        out[NSLICE * c : NSLICE * (c + 1), iperm, :] = o
    return out.reshape(B, M, I, DIM)


# revision 24
# speedup vs baseline: 1.0347x; 1.0235x over previous
"""Sparse (per-query memory) attention kernel for 8 Trainium2 NeuronCores.

Problem shapes (hardcoded):
  x    [2, 8, 128, 512] f32
  mems [2, 8, 128, 64, 512] f32
  mask [2, 8, 128, 64] bool
  Wq [512, 512], Wkv [512, 1024], Wo [512, 512], bo [512]

Sharding: pure data-parallel over the 16 (b, m) slices -> 2 slices/core.

Restructured algorithm (eliminates the kv projection):
  p[i,h,:] = Wk_h @ (scale * Wq_h^T x[i])        (key-space query vector)
  sim[i,h,j] = mems[i,j,:] . p[i,h,:]
  attn = softmax_j(sim) with 0/1 mask applied post-exp
  mbar[i,h,:] = sum_j attn[i,h,j] * mems[i,j,:]
  out = (mbar . Wv_h per head) @ Wo + bo

v3 device mapping:
  - all projections computed directly in transposed orientation (no DMA
    transposes); attn / mbar flipped via TensorE transpose-mode.
  - scores: fp8 DoubleRow outer-product per 16-query block, psum/stationary
    rows ordered (h, iq); per-ib-pair mt8 chunks so scores start early.
  - score block-diagonal extracted with 16 partition-strided SBUF->SBUF
    DMAs per slice, split over the scalar HWDGE and gpsimd SWDGE queues
    (the sync ring carries only bulk input/output traffic).
  - softmax: one ACT exp, post-exp 0/1 mask multiply + Z reduce + recip.
  - mbar: block-diag stationary windows (pre-zeroed, persistent), 8
    accumulating [128x512] matmuls per 16-query group.
  - issue order A0 X0 A1 B0 X1 B1 keeps the PE queue dense: slice 1's
    scores overlap slice 0's softmax; slice 0's mbar overlaps slice 1's.
"""

import sys

sys.path.insert(0, "/opt/trn_rl_repo")

import numpy as np
import ml_dtypes

B, M, I, J = 2, 8, 128, 64
DIM, HEADS, DIM_HEAD = 512, 8, 64
INNER = HEADS * DIM_HEAD
SCALE = DIM_HEAD**-0.5
NCORES = 8
NSLICE = (B * M) // NCORES  # slices per core = 2
PS = 128.0  # fp8 p-vector pre-scale (keeps p out of e4m3 subnormals)

TRACE = False
last_results = None

_cache = {}


def _bc(ap, pos, count):
    """Insert a stride-0 (broadcast) dim of `count` at free position `pos`."""
    import concourse.bass as bass

    l = [list(d) for d in ap.ap]
    l.insert(pos, [0, count])
    return bass.AP(tensor=ap.tensor, offset=ap.offset, ap=l)


def _ap(ap, off, dims):
    """Custom AP on the same tensor: free dims [[step,count],...] after the
    partition dim, offset in elements relative to ap's offset."""
    import concourse.bass as bass

    l = [list(ap.ap[0])] + [list(d) for d in dims]
    return bass.AP(tensor=ap.tensor, offset=ap.offset + off, ap=l)


def _pstride(ap, p0, cnt, pstep, off, dims):
    """Partition-strided slice: partitions p0, p0+pstep, ... (cnt of them),
    plus free dims and element offset."""
    import concourse.bass as bass

    l = [list(d) for d in ap.ap]
    step = l[0][0]
    part = [step * pstep, cnt]
    return bass.AP(
        tensor=ap.tensor,
        offset=ap.offset + p0 * step + off,
        ap=[part] + [list(d) for d in dims],
    )


def bass_slice_part(ap, p0, cnt):
    """Slice partitions [p0, p0+cnt) of a 2D-ish AP."""
    import concourse.bass as bass

    l = [list(d) for d in ap.ap]
    step = l[0][0]
    l[0] = [step, cnt]
    return bass.AP(tensor=ap.tensor, offset=ap.offset + p0 * step, ap=l)


def _build():
    import concourse.tile as tile
    from concourse import bacc, mybir
    import concourse.bass as bass
    from contextlib import ExitStack

    f32 = mybir.dt.float32
    bf16 = mybir.dt.bfloat16
    f8 = mybir.dt.float8e4
    Exp = mybir.ActivationFunctionType.Exp
    DR = mybir.MatmulPerfMode.DoubleRow
    AxX = mybir.AxisListType.X

    nc = bacc.Bacc("TRN2", target_bir_lowering=False, debug=False, num_devices=NCORES)

    mt8_d = nc.dram_tensor("mt8", [NSLICE * 128, 32768], f8, kind="ExternalInput")
    mje_d = nc.dram_tensor("mje", [NSLICE * 128, 32768], bf16, kind="ExternalInput")
    xt_d = nc.dram_tensor("xt", [NSLICE * DIM, I], bf16, kind="ExternalInput")
    mk_d = nc.dram_tensor("mk", [NSLICE * 128, 512], bf16, kind="ExternalInput")
    wq_d = nc.dram_tensor("wq", [DIM, INNER], bf16, kind="ExternalInput")
    wkt_d = nc.dram_tensor("wkt", [128, 2048], bf16, kind="ExternalInput")
    wv_d = nc.dram_tensor("wv", [128, 2048], bf16, kind="ExternalInput")
    wo_d = nc.dram_tensor("wo", [INNER, DIM], bf16, kind="ExternalInput")
    bo_d = nc.dram_tensor("bo", [1, DIM], f32, kind="ExternalInput")
    id_d = nc.dram_tensor("ident", [128, 128], bf16, kind="ExternalInput")
    out_d = nc.dram_tensor("out", [NSLICE * I, DIM], f32, kind="ExternalOutput")

    with tile.TileContext(nc) as tc, ExitStack() as ctx:
        const = ctx.enter_context(tc.tile_pool(name="const", bufs=1))
        mt_pool = ctx.enter_context(tc.tile_pool(name="mt", bufs=1))
        mje_pool = ctx.enter_context(tc.tile_pool(name="mje", bufs=4))
        mbT_pool = ctx.enter_context(tc.tile_pool(name="mbT", bufs=1))
        scE_pool = ctx.enter_context(tc.tile_pool(name="scE", bufs=2))
        work = ctx.enter_context(tc.tile_pool(name="work", bufs=2))
        ps_sc = ctx.enter_context(tc.tile_pool(name="pssc", bufs=2, space="PSUM"))
        ps_mb = ctx.enter_context(tc.tile_pool(name="psmb", bufs=2, space="PSUM"))
        ps_misc = ctx.enter_context(tc.tile_pool(name="psmisc", bufs=2, space="PSUM"))
        ps_pT = ctx.enter_context(tc.tile_pool(name="pspT", bufs=2, space="PSUM"))

        # --- constant weights ---
        wq_sb = const.tile([128, 4, INNER], bf16)
        nc.sync.dma_start(out=wq_sb, in_=wq_d[:, :].rearrange("(c p) n -> p c n", p=128))
        wkt_sb = const.tile([128, 4, 512], bf16)
        nc.sync.dma_start(out=wkt_sb, in_=wkt_d[:, :].rearrange("p (c n) -> p c n", c=4))
        def load_late_weights():
            # deferred until after slice 0's inputs so scores start early
            wv_sb = const.tile([128, 4, 8, 64], bf16)
            nc.sync.dma_start(
                out=wv_sb, in_=wv_d[:, :].rearrange("p (c h n) -> p c h n", c=4, h=8)
            )
            wo_sb = const.tile([128, 4, DIM], bf16)
            nc.sync.dma_start(
                out=wo_sb, in_=wo_d[:, :].rearrange("(c p) n -> p c n", p=128)
            )
            bo_sb = const.tile([128, DIM], f32)
            nc.sync.dma_start(
                out=bo_sb,
                in_=_ap(bo_d[:, :], 0, [[1, DIM]]).to_broadcast([128, DIM]),
            )
            id_sb = const.tile([128, 128], bf16)
            nc.sync.dma_start(out=id_sb, in_=id_d[:, :])
            return wv_sb, wo_sb, bo_sb, id_sb

        # persistent zeroed at3 windows [128=(e2,j64), 8 l-windows * 128]
        at3s = []
        for g in range(8):
            t = const.tile([128, 1024], bf16, tag=f"at3_{g}")
            nc.gpsimd.memset(t, 0)
            at3s.append(t)

        st = [dict() for _ in range(NSLICE)]  # per-slice tile handles

        def phase_A(s):
            """input DMAs, q^T, p^T, scores -> scE."""
            d = st[s]
            xt_sb = work.tile([128, 4, I], bf16, tag="xt")
            nc.sync.dma_start(
                out=xt_sb,
                in_=xt_d[s * DIM : (s + 1) * DIM, :].rearrange(
                    "(c p) i -> p c i", p=128
                ),
            )
            mk_sb = work.tile([128, 512], bf16, tag="mk")
            nc.sync.dma_start(out=mk_sb, in_=mk_d[s * 128 : (s + 1) * 128, :])
            d["mk"] = mk_sb
            mt8c = []
            for c in range(4):
                t = mt_pool.tile([128, 8192], f8, tag=f"mt8_{c}")
                nc.sync.dma_start(
                    out=t,
                    in_=mt8_d[s * 128 : (s + 1) * 128, c * 8192 : (c + 1) * 8192],
                )
                mt8c.append(t)

            # q^T directly: qt[(hp,dh), (c,i)]
            qt_ps = ps_misc.tile([128, 512], f32, tag="misc")
            for c in range(4):
                for dc in range(4):
                    nc.tensor.matmul(
                        qt_ps[:, c * 128 : (c + 1) * 128],
                        wq_sb[:, dc, c * 128 : (c + 1) * 128],
                        xt_sb[:, dc, :],
                        start=(dc == 0),
                        stop=(dc == 3),
                    )
            qt_sb = work.tile([128, 4, I], bf16, tag="qt")
            nc.vector.tensor_copy(
                out=qt_sb[:, :, :].rearrange("p a b -> p (a b)"), in_=qt_ps
            )

            # p^T = (Wk_h * PS) @ q_h, cast fp8. pt8 free: (dc4, ib8, h8, iq16)
            pt8 = work.tile([128, 4096], f8, tag="pt8")
            for h in range(8):
                pp = ps_misc.tile([128, 512], f32, tag="misc")
                hp = h % 2
                for dc in range(4):
                    nc.tensor.matmul(
                        pp[:, dc * 128 : (dc + 1) * 128],
                        wkt_sb[
                            hp * 64 : hp * 64 + 64, h // 2, dc * 128 : (dc + 1) * 128
                        ],
                        qt_sb[hp * 64 : hp * 64 + 64, h // 2, :],
                        start=True,
                        stop=True,
                    )
                # dst col = dc*1024 + ib*128 + h*16 + iq  (contiguous 16-runs)
                dst = _ap(pt8[:, :], h * 16, [[1024, 4], [128, 8], [1, 16]])
                src = _ap(pp[:, :], 0, [[128, 4], [16, 8], [1, 16]])
                if h % 2 == 0:
                    nc.vector.tensor_copy(out=dst, in_=src)
                else:
                    nc.scalar.copy(out=dst, in_=src)

            # scores (fp8 DoubleRow): psum rows (h8, iq16), hf-major so the
            # diag extraction of each half starts at half-time.
            # scE half-tile hf: [128, (ib8, j64x8q)]; extraction DMAs inline.
            sim = work.tile([128, 512], f32, tag="sim")
            d["sim"] = sim
            for hf in range(2):
                scE = scE_pool.tile([128, 4096], f32, tag=f"scE{hf}")
                for ib in range(8):
                    ibp, e = ib // 2, ib % 2
                    sc = ps_sc.tile([128, 512], f32, tag="sc")
                    for p in range(2):
                        stat = _ap(
                            pt8[:, :], p * 2048 + ib * 128, [[1024, 2], [1, 128]]
                        )
                        mov = _ap(
                            mt8c[ibp][:, :],
                            p * 4096 + e * 1024 + hf * 512,
                            [[2048, 2], [1, 512]],
                        )
                        nc.tensor.matmul(
                            sc,
                            stat,
                            mov,
                            start=(p == 0),
                            stop=(p == 1),
                            perf_mode=DR,
                            skip_group_check=True,
                        )
                    dst = scE[:, ib * 512 : (ib + 1) * 512]
                    if ib % 3 == 2:
                        nc.scalar.copy(out=dst, in_=sc)
                    else:
                        nc.vector.tensor_copy(out=dst, in_=sc)
                # extraction: queries iq = hf*8 + q, q in [0,8)
                for q in range(8):
                    iq = hf * 8 + q
                    src = _pstride(
                        scE[:, :], iq, 8, 16, q * 64, [[512, 8], [1, 64]]
                    )
                    dstx = _pstride(sim[:, :], iq, 8, 16, 0, [[64, 8], [1, 64]])
                    eng = (nc.scalar, nc.gpsimd, nc.sync)[q % 3]
                    eng.dma_start(out=dstx, in_=src)

        def phase_X(s):
            """softmax (extraction DMAs were issued inline in phase A)."""
            d = st[s]
            sim, mk_sb = d["sim"], d["mk"]
            ex = work.tile([128, 512], bf16, tag="ex")
            nc.scalar.activation(out=ex, in_=sim, func=Exp, bias=0.0, scale=1.0 / PS)
            em = work.tile([128, 8, 64], bf16, tag="em")
            nc.vector.tensor_mul(
                em[:, :, :].rearrange("p a b -> p (a b)"), ex, mk_sb
            )
            zs = work.tile([128, 8], f32, tag="zs")
            nc.vector.reduce_sum(zs, em, axis=AxX)
            rz = work.tile([128, 8], f32, tag="rz")
            nc.vector.reciprocal(rz, zs)
            rzb = work.tile([128, 8], bf16, tag="rzb")
            nc.vector.tensor_copy(out=rzb, in_=rz)
            attn = work.tile([128, 8, 64], bf16, tag="attn")
            nc.vector.tensor_mul(attn, em, _bc(rzb[:, :], 2, 64))
            d["attn"] = attn

        def phase_B(s):
            """attn^T, scatter, mbar(+mje DMA), mbar^T, o1, out."""
            d = st[s]
            attn = d["attn"]
            atT = ps_pT.tile([128, 512], bf16, tag="pT")
            for a in range(4):
                nc.tensor.transpose(
                    atT[:, a * 128 : (a + 1) * 128],
                    attn[:, 2 * a : 2 * a + 2, :].rearrange("p a b -> p (a b)"),
                    id_sb,
                )
            # scatter: src col = a*128 + h*16 + half*8 + l; dst l*144 + h*2 + e
            for g in range(8):
                a, half = g // 2, g % 2
                for e in range(2):
                    src = _ap(
                        bass_slice_part(atT[:, :], e * 64, 64),
                        a * 128 + half * 8,
                        [[1, 8], [16, 8]],
                    )
                    dst = _ap(
                        bass_slice_part(at3s[g][:, :], e * 64, 64),
                        e,
                        [[144, 8], [2, 8]],
                    )
                    nc.vector.tensor_copy(out=dst, in_=src)

            # mbar: psum partition p = l*16 + h*2 + e
            mbT_sb = mbT_pool.tile([128, 4, 1024], bf16, tag="mbT")
            mjes = []
            for g in range(8):
                mje_sb = mje_pool.tile([128, 4096], bf16, tag="mje")
                nc.sync.dma_start(
                    out=mje_sb,
                    in_=mje_d[s * 128 : (s + 1) * 128, g * 4096 : (g + 1) * 4096],
                )
                mjes.append(mje_sb)
            # one-group lag: group g's transposes issue after group g+1's
            # matmuls so the cross-engine mbE evict overlaps PE work.
            mbEs = {}
            for g in range(9):
                if g < 8:
                    mje_sb = mjes[g]
                    mb = ps_mb.tile([128, 512], f32, tag="mb")
                    for l in range(8):
                        nc.tensor.matmul(
                            mb,
                            at3s[g][:, l * 128 : l * 128 + 128],
                            mje_sb[:, l * 512 : (l + 1) * 512],
                            start=(l == 0),
                            stop=(l == 7),
                        )
                    mbE = work.tile([128, 512], bf16, tag="mbE")
                    if g % 2 == 0:
                        nc.vector.tensor_copy(out=mbE, in_=mb)
                    else:
                        nc.scalar.copy(out=mbE, in_=mb)
                    mbEs[g] = mbE
                if g == 0:
                    continue
                gl = g - 1
                mbE = mbEs.pop(gl)
                mbT_ps = ps_pT.tile([128, 512], bf16, tag="pT")
                for dc in range(4):
                    nc.tensor.transpose(
                        mbT_ps[:, dc * 128 : (dc + 1) * 128],
                        mbE[:, dc * 128 : (dc + 1) * 128],
                        id_sb,
                    )
                # dst col = dc*1024 + h*128 + gl*16 + l*2 + e
                for e in range(2):
                    src = _ap(mbT_ps[:, :], e, [[128, 4], [16, 8], [2, 8]])
                    dst = _ap(
                        mbT_sb[:, :, :].rearrange("p a b -> p (a b)"),
                        gl * 16 + e,
                        [[1024, 4], [2, 8], [128, 8]],
                    )
                    if (gl + e) % 2 == 0:
                        nc.scalar.copy(out=dst, in_=src)
                    else:
                        nc.vector.tensor_copy(out=dst, in_=src)

            # o1^T: [(hp,dh), (c, m)]
            o1_ps = ps_misc.tile([128, 512], f32, tag="misc")
            for h in range(8):
                c, hp = h // 2, h % 2
                for dc in range(4):
                    nc.tensor.matmul(
                        o1_ps[hp * 64 : hp * 64 + 64, c * 128 : (c + 1) * 128],
                        wv_sb[:, dc, h, :],
                        mbT_sb[:, dc, h * 128 : (h + 1) * 128],
                        start=(dc == 0),
                        stop=(dc == 3),
                    )
            o1_sb = work.tile([128, 4, I], bf16, tag="o1")
            nc.vector.tensor_copy(
                out=o1_sb[:, :, :].rearrange("p a b -> p (a b)"), in_=o1_ps
            )

            fin = ps_mb.tile([128, DIM], f32, tag="mb")
            for c in range(4):
                nc.tensor.matmul(
                    fin, o1_sb[:, c, :], wo_sb[:, c, :], start=(c == 0), stop=(c == 3)
                )
            outb = work.tile([128, DIM], f32, tag="outb")
            nc.vector.tensor_add(outb, fin, bo_sb)
            nc.sync.dma_start(out=out_d[s * I : (s + 1) * I, :], in_=outb)

        phase_A(0)
        wv_sb, wo_sb, bo_sb, id_sb = load_late_weights()
        phase_A(1)
        phase_X(0)
        phase_X(1)
        phase_B(0)
        phase_B(1)

    nc.compile()
    return nc


def kernel(x, mems, mask, Wq, Wkv, Wo, bo):
    from concourse.bass_utils import run_bass_kernel_spmd

    global last_results

    if "nc" not in _cache:
        _cache["nc"] = _build()
    nc = _cache["nc"]

    bf = ml_dtypes.bfloat16
    f8 = ml_dtypes.float8_e4m3
    S = B * M
    x = np.asarray(x, dtype=np.float32).reshape(S, I, DIM)
    mems = np.asarray(mems, dtype=np.float32).reshape(S, I, J, DIM)
    mask = np.asarray(mask).reshape(S, I, J)

    # memsT8 fp8: [dsub128, ibp4, pass2, k2, e2, iq16, j64]
    mt8 = (
        mems.astype(f8)
        .reshape(S, 4, 2, 16, J, 2, 2, 128)  # [S, ibp, e, iq, j, pass, k, dsub]
        .transpose(0, 7, 1, 5, 6, 2, 3, 4)  # [S, dsub, ibp, pass, k, e, iq, j]
        .reshape(S, 128, 32768)
    )
    # memsJE bf16 rows (e2,j64), cols (a4, half2, l8, d): i = a*32+e*16+half*8+l
    mje = (
        mems.astype(bf)
        .reshape(S, 4, 2, 2, 8, J, DIM)  # [S, a, e, half, l, j, d]
        .transpose(0, 2, 5, 1, 3, 4, 6)  # [S, e, j, a, half, l, d]
        .reshape(S, 128, 32768)
    )
    # xT bf16 [d, i]
    xt = np.ascontiguousarray(x.transpose(0, 2, 1)).astype(bf)
    # mask 0/1 bf16: rows (h8, iq16), cols (ib8, j64)
    m1 = mask.reshape(S, 8, 16, J).transpose(0, 2, 1, 3)  # [S, iq, ib, j]
    mk = np.broadcast_to(m1[:, None, :, :, :], (S, 8, 16, 8, J)).reshape(S, 128, 512)
    mk = np.ascontiguousarray(mk).astype(bf)

    Wq_r = (np.asarray(Wq, np.float32) * SCALE).astype(bf)
    Wkv = np.asarray(Wkv, np.float32)
    Wk, Wv = Wkv[:, :INNER], Wkv[:, INNER:]
    # WkT xPS: [(hp2, dh64), c4, D512]: row hp*64+dh, chunk c -> head 2c+hp
    wkt = (
        (Wk * PS)
        .reshape(DIM, 4, 2, 64)  # [D, c, hp, dh]
        .transpose(2, 3, 1, 0)  # [hp, dh, c, D]
        .reshape(128, 2048)
    )
    wkt = np.ascontiguousarray(wkt).astype(bf)
    # Wv: [dsub128, dc4, h8, dh64]
    wv = np.ascontiguousarray(
        Wv.reshape(4, 128, 8, 64).transpose(1, 0, 2, 3)
    ).astype(bf).reshape(128, 2048)
    wo_r = np.asarray(Wo, np.float32).astype(bf)
    bo_r = np.asarray(bo, np.float32).reshape(1, DIM)
    ident = np.eye(128, dtype=bf)

    in_maps = []
    for c in range(NCORES):
        sl = slice(NSLICE * c, NSLICE * (c + 1))
        in_maps.append(
            {
                "mt8": mt8[sl].reshape(NSLICE * 128, 32768),
                "mje": mje[sl].reshape(NSLICE * 128, 32768),
                "xt": xt[sl].reshape(NSLICE * DIM, I),
                "mk": mk[sl].reshape(NSLICE * 128, 512),
                "wq": Wq_r,
                "wkt": wkt,
                "wv": wv,
                "wo": wo_r,
                "bo": bo_r,
                "ident": ident,
            }
        )

    res = run_bass_kernel_spmd(nc, in_maps, core_ids=list(range(NCORES)), trace=TRACE)
    last_results = res

    # device row order m = g*16 + l*2 + e, g = a*2+half -> i = a*32+e*16+half*8+l
    mm = np.arange(128)
    g, le = mm // 16, mm % 16
    l, e = le // 2, le % 2
    a, half = g // 2, g % 2
    iperm = a * 32 + e * 16 + half * 8 + l  # iperm[m] = original query row i

    out = np.empty((S, I, DIM), np.float32)
    for c in range(NCORES):
        o = res.results[c]["out"].reshape(NSLICE, I, DIM)
        out[NSLICE * c : NSLICE * (c + 1), iperm, :] = o
    return out.reshape(B, M, I, DIM)
